# revision 3
# baseline (speedup 1.0000x reference)
"""Trainium2 Bass kernel v2 for nn_ChannelSegment (differential-attention MoE).

Sharding: 8 cores = 4 channels x 2 batches; core i handles (b, n) = (i//4, i%4).

v2 layout strategy vs v1:
- PV matmuls run with P chunks stationary and V (+ones col) moving, producing
  attention outputs in [query-partition, head-dim] layout. Softmax
  denominators land as per-partition scalar columns, so the branch combine
  u = o1*den2 - lam*o2*den1 is a few large DVE ops with broadcast APs
  instead of per-(h,c) PE broadcasts + row ops.
- Causal masks applied as one batched DVE multiply over all 8 heads per
  (key-block, branch) instead of 128 gpsimd ops.
- qkv/v biases added via rank-1 matmul accumulation (ones (x) bias row).
- wh*0.8 diff-rms weight folded into Wout host-side; routing weight folded
  into a host-prescaled residual copy and the final rsqrt scale.
- diff tiles transposed back to [feature, token] with PE transposes.
- Emission interleaves second-half scores with first-half PV to keep the
  PE array continuously busy (HAM clock stays warm).
"""
import os
import sys

sys.path.insert(0, "/opt/trn_rl_repo")

import numpy as np
import ml_dtypes

from concourse import bacc
import concourse.tile as tile
from concourse import mybir
from concourse.bass_utils import run_bass_kernel_spmd

N_CH, CW, H, D, D2 = 4, 512, 8, 64, 32
L, B = 1024, 2
EPS = 1e-6
LAM0 = 0.2
SCALE = float(1.0 / np.sqrt(np.float32(D2)))

F32 = mybir.dt.float32
F32R = mybir.dt.float32r
BF16 = mybir.dt.bfloat16
AF = mybir.ActivationFunctionType
OP = mybir.AluOpType

_cache = {}

# packed p-tile column offsets per key-block t, and block widths
OFF0 = [0, 512, 896, 1152]          # c=0: widths 512,384,256,128 -> 1280 cols
W0 = [512, 384, 256, 128]
OFF1 = [0, 512, 1024, 1536, 2048, 2560, 2944, 3200]   # c=1 -> 3328 cols
W1 = [512, 512, 512, 512, 512, 384, 256, 128]
P0_COLS = 1280
P1_COLS = 3328


def _build():
    from contextlib import ExitStack

    nc = bacc.Bacc("TRN2", target_bir_lowering=False, num_devices=8)

    dp = nc.declare_dram_parameter
    hT_d = dp("hT", [CW, L], F32R, isOutput=False)
    hTw_d = dp("hTw", [CW, L], F32, isOutput=False)
    wqk_d = dp("wqk", [CW, 2 * CW], F32R, isOutput=False)
    wv_d = dp("wv", [CW, CW], F32R, isOutput=False)
    wout_d = dp("wout", [CW, CW], BF16, isOutput=False)
    bqk_d = dp("bqk", [8, 128, 1], F32, isOutput=False)
    bv_d = dp("bv", [1, CW], F32R, isOutput=False)
    bout_d = dp("bout", [4, 128, 1], F32, isOutput=False)
    qmul_d = dp("qmul", [4, 128, 1], F32, isOutput=False)
    wn_d = dp("wn", [4, 128, 1], F32, isOutput=False)
    lamn_d = dp("lamn", [128, 1], F32, isOutput=False)
    epsw2_d = dp("epsw2", [1, 1], F32, isOutput=False)
    rfscale_d = dp("rfscale", [1, 1], F32, isOutput=False)
    tri8_d = dp("tri8", [128, 8, 128], BF16, isOutput=False)
    ident_d = dp("ident", [128, 128], BF16, isOutput=False)
    e4_d = dp("e4", [128, 4], F32R, isOutput=False)
    b4_d = dp("b4", [4, 128], F32R, isOutput=False)
    o1128_d = dp("o1128", [1, 128], F32R, isOutput=False)
    ones_d = dp("ones", [128, 1], F32R, isOutput=False)
    yT_d = dp("yT", [CW, L], F32, isOutput=True)
    debug = bool(os.environ.get("KERNEL_DEBUG"))
    if debug:
        dbg_qk_d = dp("dbg_qk", [2 * CW, L], F32, isOutput=True)
        dbg_diffT_d = dp("dbg_diffT", [CW, L], F32, isOutput=True)
        dbg_attn_d = dp("dbg_attn", [CW, L], F32, isOutput=True)

    with tile.TileContext(nc) as tc:
        est = ExitStack()
        est.enter_context(nc.allow_low_precision(reason="f32r/bf16 intermediates"))
        persist = est.enter_context(tc.tile_pool(name="persist", bufs=1))
        work = est.enter_context(tc.tile_pool(name="work", bufs=1))
        ps_mm = est.enter_context(tc.tile_pool(name="ps_mm", bufs=3, space="PSUM"))
        ps_o = est.enter_context(tc.tile_pool(name="ps_o", bufs=2, space="PSUM"))

        dma = nc.sync.dma_start

        # ---------------- persistent tiles ----------------
        qkT = [persist.tile([128, L], BF16, tag=f"qkT{j}", name=f"qkT{j}") for j in range(8)]
        v_aug = [persist.tile([128, 8, 65], BF16, tag=f"vaug{t}", name=f"vaug{t}") for t in range(8)]
        wout_sb = [persist.tile([128, CW], BF16, tag=f"wo{k}", name=f"wo{k}") for k in range(4)]
        diffT = persist.tile([128, 4, 512], BF16, tag="diffT", name="diffT")
        tri8 = persist.tile([128, 8, 128], BF16, tag="tri8", name="tri8")
        ident = persist.tile([128, 128], BF16, tag="ident", name="ident")
        o1128 = persist.tile([1, 128], F32R, tag="o1128", name="o1128")
        ones = persist.tile([128, 1], F32R, tag="ones", name="ones")
        bqk = [persist.tile([128, 1], F32, tag=f"bqk{j}", name=f"bqk{j}") for j in range(8)]
        bout = [persist.tile([128, 1], F32, tag=f"bout{j}", name=f"bout{j}") for j in range(4)]
        qmul = [persist.tile([128, 1], F32, tag=f"qmul{j}", name=f"qmul{j}") for j in range(4)]
        wn_sb = [persist.tile([128, 1], F32, tag=f"wn{j}", name=f"wn{j}") for j in range(4)]
        lamn = persist.tile([128, 1], F32, tag="lamn", name="lamn")
        epsw2 = persist.tile([1, 1], F32, tag="epsw2", name="epsw2")
        rfscale = persist.tile([1, 1], F32, tag="rfscale", name="rfscale")
        eps_sb = persist.tile([128, 1], F32, tag="eps_sb", name="eps_sb")
        nc.vector.memset(eps_sb, EPS)
        I32 = mybir.dt.int32
        magic = persist.tile([128, 8], I32, tag="magic", name="magic")
        nc.vector.memset(magic, 0x5f3759df)

        dma(out=tri8, in_=tri8_d[:])
        dma(out=ident, in_=ident_d[:])
        dma(out=o1128, in_=o1128_d[:])
        dma(out=ones, in_=ones_d[:])
        for j in range(8):
            dma(out=bqk[j], in_=bqk_d[j])
        for j in range(4):
            dma(out=bout[j], in_=bout_d[j])
            dma(out=qmul[j], in_=qmul_d[j])
            dma(out=wn_sb[j], in_=wn_d[j])
        dma(out=lamn, in_=lamn_d[:])
        dma(out=epsw2, in_=epsw2_d[:])
        dma(out=rfscale, in_=rfscale_d[:])

        # attn output tiles (rotate over c)
        attn = [persist.tile([128, 512], BF16, tag=f"attn{j}", bufs=1, name=f"attn{j}") for j in range(4)]

        # working tiles for attention postprocess
        def wtile(shape, dt, tag, bufs=2):
            return lambda: work.tile(shape, dt, tag=tag, bufs=bufs, name=tag)

        # ---------------- phase-1 pools (released before p pools open) ----
        with tc.tile_pool(name="pw", bufs=1) as pw:
            hT = [pw.tile([128, L], F32R, tag=f"hT{k}", name=f"hT{k}") for k in range(4)]
            wqk = [pw.tile([128, 2 * CW], F32R, tag=f"wq{k}", name=f"wq{k}") for k in range(4)]
            wv = [pw.tile([128, CW], F32R, tag=f"wv{k}", name=f"wv{k}") for k in range(4)]
            e4 = pw.tile([128, 4], F32R, tag="e4", name="e4")
            b4 = pw.tile([4, 128], F32R, tag="b4", name="b4")
            bv_sb = pw.tile([1, CW], F32R, tag="bv_sb", name="bv_sb")
            dma(out=e4, in_=e4_d[:])
            dma(out=b4, in_=b4_d[:])
            dma(out=bv_sb, in_=bv_d[:])

            for k in range(4):
                dma(out=wqk[k][:, 0:512], in_=wqk_d[128 * k : 128 * (k + 1), 0:512])
                dma(out=hT[k][:, 0:512], in_=hT_d[128 * k : 128 * (k + 1), 0:512])
            for k in range(4):
                dma(out=hT[k][:, 512:1024], in_=hT_d[128 * k : 128 * (k + 1), 512:1024])
            for k in range(4):
                dma(out=wqk[k][:, 512:1024], in_=wqk_d[128 * k : 128 * (k + 1), 512:1024])
            for k in range(4):
                dma(out=wv[k], in_=wv_d[128 * k : 128 * (k + 1), :])

            # ---- PE pre-warm: dummy matmuls during input DMA wait ----
            # ~64 x 128-row identity transposes = ~3.4us sustained PE activity,
            # enough for one busy HAM window so MM1 starts at full clock.
            for i in range(16):
                wps = ps_mm.tile([128, 512], BF16, tag="mm", padded_shape=[128, 2048], name="warm")
                for g in range(4):
                    nc.tensor.transpose(
                        wps[:, 128 * g : 128 * (g + 1)], ident, ident,
                    )

            # ---- MM1a: qkT = silu(wqk.T @ hT + bqk), pair psum over c ----
            # j0/j1: c-sequential so PE starts before second-half DMAs land
            for j in range(8):
                ps = ps_mm.tile([128, 1024], F32, tag="mm", name="mm")
                if j < 2:
                    for c in range(2):
                        for k in range(4):
                            nc.tensor.matmul(
                                ps[:, 512 * c : 512 * (c + 1)],
                                wqk[k][:, 128 * j : 128 * (j + 1)],
                                hT[k][:, 512 * c : 512 * (c + 1)],
                                start=(k == 0),
                                stop=(k == 3),
                            )
                else:
                    for k in range(4):
                        nc.tensor.matmul(
                            ps[:, 0:512],
                            wqk[k][:, 128 * j : 128 * (j + 1)],
                            hT[k][:, 0:512],
                            start=(k == 0),
                            stop=(k == 3),
                        )
                        nc.tensor.matmul(
                            ps[:, 512:1024],
                            wqk[k][:, 128 * j : 128 * (j + 1)],
                            hT[k][:, 512:1024],
                            start=(k == 0),
                            stop=(k == 3),
                        )
                nc.scalar.activation(
                    out=qkT[j], in_=ps, func=AF.Silu, bias=bqk[j],
                )

            # ---- MM1b: v_aug = silu(h @ wv + bv) with ones col, pair over t ----
            for tp in range(4):
                ps = ps_mm.tile([128, 1024], F32, tag="mm", name="mm")
                for i in range(2):
                    t = 2 * tp + i
                    nc.vector.memset(v_aug[t][:, :, 64:65], 1.0)
                    sl = ps[:, 512 * i : 512 * (i + 1)]
                    for k in range(4):
                        nc.tensor.matmul(
                            sl,
                            hT[k][:, 128 * t : 128 * (t + 1)],
                            wv[k],
                            start=(k == 0),
                            stop=False,
                        )
                    nc.tensor.matmul(sl, o1128, bv_sb, start=False, stop=True)
                for i in range(2):
                    t = 2 * tp + i
                    nc.scalar.activation(
                        out=v_aug[t][:, :, 0:64],
                        in_=ps[:, 512 * i : 512 * (i + 1)].rearrange("p (h d) -> p h d", d=64),
                        func=AF.Silu,
                    )

            # ---- q/k group rms (pair psum over c) ----
            for j in range(8):
                sq = pw.tile([128, 1024], F32R, tag="sq", bufs=2, name="sq")
                if j % 2 == 0:
                    nc.vector.tensor_mul(out=sq, in0=qkT[j], in1=qkT[j])
                else:
                    nc.gpsimd.tensor_mul(out=sq, in0=qkT[j], in1=qkT[j])
                msq = ps_mm.tile([128, 1024], F32, tag="mm", name="mm")
                nc.tensor.matmul(msq[0:4, 0:512], e4, sq[:, 0:512], start=True, stop=True)
                nc.tensor.matmul(msq[0:4, 512:1024], e4, sq[:, 512:1024], start=True, stop=True)
                rall = pw.tile([128, 1024], F32R, tag="rall", bufs=2, name="rall")
                nc.scalar.activation(
                    out=rall[0:4, :], in_=msq[0:4, :], func=AF.Abs_reciprocal_sqrt,
                    scale=1.0 / 32.0, bias=eps_sb[0:4, :],
                )
                rbc = ps_mm.tile([128, 1024], F32, tag="mm", name="mm")
                nc.tensor.matmul(rbc[:, 0:512], b4, rall[0:4, 0:512], start=True, stop=True)
                nc.tensor.matmul(rbc[:, 512:1024], b4, rall[0:4, 512:1024], start=True, stop=True)
                for c in range(2):
                    if j < 4:
                        nc.vector.scalar_tensor_tensor(
                            out=qkT[j][:, 512 * c : 512 * (c + 1)],
                            in0=qkT[j][:, 512 * c : 512 * (c + 1)],
                            scalar=qmul[j],
                            in1=rbc[:, 512 * c : 512 * (c + 1)],
                            op0=OP.mult,
                            op1=OP.mult,
                        )
                    else:
                        nc.vector.tensor_mul(
                            out=qkT[j][:, 512 * c : 512 * (c + 1)],
                            in0=qkT[j][:, 512 * c : 512 * (c + 1)],
                            in1=rbc[:, 512 * c : 512 * (c + 1)],
                        )

        for k in range(4):
            dma(out=wout_sb[k], in_=wout_d[128 * k : 128 * (k + 1), :])
        if debug:
            for j in range(8):
                dma(out=dbg_qk_d[128 * j : 128 * (j + 1), :], in_=qkT[j].bitcast(F32))

        # ---------------- attention ----------------
        p0 = None
        p1 = None

        def emit_scores(pt, c, t):
            """Scores+exp for key block t into packed p tile pt (half c)."""
            off = (OFF0 if c == 0 else OFF1)[t]
            w = (W0 if c == 0 else W1)[t]
            q_lo = 512 * c + (512 - w)
            for h in range(8):
                jq = h // 2
                jk = 4 + h // 2
                po = 64 * (h % 2)
                s_ps = ps_mm.tile([128, 1024], F32, tag="mm", name="mm")
                for br in range(2):
                    bo = po + 32 * br
                    nc.tensor.matmul(
                        s_ps[:, 512 * br : 512 * br + w],
                        qkT[jk][bo : bo + 32, 128 * t : 128 * (t + 1)],
                        qkT[jq][bo : bo + 32, q_lo : 512 * (c + 1)],
                        start=True,
                        stop=True,
                        tile_position=(bo, 0),
                    )
                nc.scalar.activation(
                    out=pt[:, h, :, off : off + w],
                    in_=s_ps.rearrange("p (b x) -> p b x", b=2)[:, :, 0:w],
                    func=AF.Exp, scale=SCALE,
                )
            if t >= 4 * c:  # diagonal block: causal mask, batched over heads
                for br in range(2):
                    sl = pt[:, :, br, off : off + 128]
                    nc.vector.tensor_mul(out=sl, in0=sl, in1=tri8)

        def p_chunk(pt, c, h, br, t, qb):
            off = (OFF0 if c == 0 else OFF1)[t]
            w = (W0 if c == 0 else W1)[t]
            lo = off + 128 * qb - (512 * (c + 1) - w)  # qb col within packed block
            return pt[:, h, br, lo : lo + 128]

        def emit_pv_qb(pt, c, qb):
            """PV + postprocess + transposes for query block qb (global)."""
            u = work.tile([128, 8, 64], F32, tag="u", bufs=2, name="u")
            dens = work.tile([128, 2, 2, 4], F32, tag="dens", bufs=2, name="dens")
            for half in range(2):
                o_ps = {}
                for br in range(2):
                    o_ps[br] = ps_o.tile([128, 260], F32, tag="o", name="o")
                for br in range(2):
                    for hh in range(4):
                        h = 4 * half + hh
                        for kb in range(qb + 1):
                            nc.tensor.matmul(
                                o_ps[br][:, 65 * hh : 65 * hh + 65],
                                p_chunk(pt, c, h, br, kb, qb),
                                v_aug[kb][:, h, :],
                                start=(kb == 0),
                                stop=(kb == qb),
                            )
                for br in range(2):
                    nc.vector.tensor_copy(
                        out=dens[:, half, br, :],
                        in_=o_ps[br].rearrange("p (h x) -> p h x", x=65)[:, :, 64],
                    )
                oa = o_ps[0].rearrange("p (h x) -> p h x", x=65)
                ob = o_ps[1].rearrange("p (h x) -> p h x", x=65)
                u_sl = u[:, 4 * half : 4 * half + 4, :]
                t2 = work.tile([128, 4, 64], F32, tag="t2", bufs=2, name="t2")
                nc.vector.tensor_mul(
                    out=u_sl, in0=oa[:, :, 0:64],
                    in1=dens[:, half, 1, :].rearrange("p (h o) -> p h o", o=1).to_broadcast([128, 4, 64]),
                )
                nc.vector.tensor_mul(
                    out=t2, in0=ob[:, :, 0:64],
                    in1=dens[:, half, 0, :].rearrange("p (h o) -> p h o", o=1).to_broadcast([128, 4, 64]),
                )
                nc.vector.scalar_tensor_tensor(
                    out=u_sl,
                    in0=t2, scalar=lamn, in1=u_sl,
                    op0=OP.mult, op1=OP.add,
                )
            usq = work.tile([128, 8, 64], BF16, tag="usq", bufs=2, name="usq")
            nc.gpsimd.tensor_mul(out=usq, in0=u, in1=u)
            msq = work.tile([128, 8], F32, tag="msq", bufs=2, name="msq")
            nc.vector.tensor_reduce(
                out=msq.rearrange("p (h o) -> p h o", o=1), in_=usq,
                axis=mybir.AxisListType.X, op=OP.add,
            )
            # fast inverse sqrt on DVE: rt = 8/sqrt(msq)  (folds the 1/64)
            sh = work.tile([128, 8], I32, tag="sh", bufs=2, name="sh")
            nc.vector.tensor_scalar(
                out=sh, in0=msq.bitcast(I32), scalar1=1, scalar2=None,
                op0=OP.logical_shift_right,
            )
            nc.vector.tensor_sub(out=sh, in0=magic, in1=sh)
            y0 = sh.bitcast(F32)
            a = work.tile([128, 8], F32, tag="nra", bufs=2, name="nra")
            nc.vector.tensor_mul(out=a, in0=y0, in1=y0)
            nc.vector.tensor_mul(out=a, in0=a, in1=msq)
            nc.vector.tensor_scalar(
                out=a, in0=a, scalar1=-4.0, scalar2=12.0, op0=OP.mult, op1=OP.add,
            )
            rt = work.tile([128, 8], F32, tag="rt", bufs=2, name="rt")
            nc.vector.tensor_mul(out=rt, in0=y0, in1=a)
            u2 = work.tile([128, 8, 64], BF16, tag="usq", bufs=2, name="u2")
            nc.vector.tensor_mul(
                out=u2, in0=u,
                in1=rt.rearrange("p (h o) -> p h o", o=1).to_broadcast([128, 8, 64]),
            )
            return (u2, qb)

        def emit_postpe(u2, qb):
            u2f = u2.rearrange("p h d -> p (h d)")
            psT = ps_mm.tile([128, 512], BF16, tag="mm", padded_shape=[128, 2048], name="mm")
            for g in range(4):
                nc.tensor.transpose(
                    psT[:, 128 * g : 128 * (g + 1)],
                    u2f[:, 128 * g : 128 * (g + 1)],
                    ident,
                )
            qo = qb % 4
            nc.vector.tensor_copy(
                out=diffT[:, :, 128 * qo : 128 * (qo + 1)],
                in_=psT.rearrange("p (k x) -> p k x", x=128),
            )

        def emit_mm2_final(c):
            fin = ps_mm.tile([128, 1024], F32, tag="mm", name="mm")
            for j in range(4):
                ps = ps_mm.tile([128, 1024], F32, tag="mm", name="mm")
                for k in range(4):
                    nc.tensor.matmul(
                        ps[:, 0:512],
                        wout_sb[k][:, 128 * j : 128 * (j + 1)],
                        diffT[:, k, :],
                        start=(k == 0),
                        stop=(k == 3),
                    )
                nc.scalar.activation(
                    out=attn[j], in_=ps[:, 0:512], func=AF.Silu, bias=bout[j],
                )
                asq = work.tile([128, 512], F32R, tag="asq", bufs=2, name="asq")
                nc.vector.tensor_mul(out=asq, in0=attn[j], in1=attn[j])
                nc.tensor.matmul(fin[0:1, 0:512], ones, asq, start=(j == 0), stop=(j == 3))
            if debug:
                for k in range(4):
                    dma(
                        out=dbg_diffT_d[128 * k : 128 * (k + 1), 512 * c : 512 * (c + 1)],
                        in_=diffT[:, k, :].bitcast(F32)[:, 0:256],
                    )
            rf = work.tile([1, 512], F32R, tag="rf", bufs=1, name="rf")
            nc.scalar.activation(
                out=rf, in_=fin[0:1, 0:512], func=AF.Abs_reciprocal_sqrt,
                scale=rfscale, bias=epsw2,
            )
            rfbc = ps_mm.tile([128, 1024], F32, tag="mm", name="mm")
            nc.tensor.matmul(rfbc[:, 0:512], o1128, rf, start=True, stop=True)
            for j in range(4):
                hres = work.tile([128, 512], F32, tag="hres", bufs=2, name="hres")
                dma(out=hres, in_=hTw_d[128 * j : 128 * (j + 1), 512 * c : 512 * (c + 1)])
                ytmp = work.tile([128, 512], F32, tag="ytmp", bufs=2, name="ytmp")
                nc.vector.tensor_mul(out=ytmp, in0=attn[j], in1=rfbc[:, 0:512])
                nc.vector.scalar_tensor_tensor(
                    out=ytmp, in0=ytmp, scalar=wn_sb[j], in1=hres,
                    op0=OP.mult, op1=OP.add,
                )
                if debug:
                    dma(
                        out=dbg_attn_d[128 * j : 128 * (j + 1), 512 * c : 512 * (c + 1)],
                        in_=attn[j].bitcast(F32)[:, 0:256],
                    )
                dma(
                    out=yT_d[128 * j : 128 * (j + 1), 512 * c : 512 * (c + 1)],
                    in_=ytmp,
                )

        with tc.tile_pool(name="pp0", bufs=1) as pp0:
            p0 = pp0.tile([128, 8, 2, P0_COLS], BF16, tag="p0", name="p0")
            with tc.tile_pool(name="pp1", bufs=1) as pp1:
                p1 = pp1.tile([128, 8, 2, P1_COLS], BF16, tag="p1", name="p1")

                # c=0 scores
                for t in range(4):
                    emit_scores(p0, 0, t)
                # interleave: c=1 scores with c=0 PV; transposes lag one step
                pend = None
                for t in range(4):
                    emit_scores(p1, 1, t)
                    if pend is not None:
                        emit_postpe(*pend)
                    pend = emit_pv_qb(p0, 0, t)
                emit_scores(p1, 1, 4)
                emit_postpe(*pend)
                emit_mm2_final(0)
                pend = emit_pv_qb(p1, 1, 4)
                for t in range(5, 8):
                    emit_scores(p1, 1, t)
                    emit_postpe(*pend)
                    pend = emit_pv_qb(p1, 1, t)
                emit_postpe(*pend)
                emit_mm2_final(1)
        est.close()

    nc.compile()
    return nc


def make_in_maps(x, routing_weights, Wqkv, bqkv, Wout, bout_a, lq1, lk1, lq2, lk2, wq, wk, wh, wn):
    tri8 = np.broadcast_to(
        np.triu(np.ones((128, 128), np.float32))[:, None, :], (128, 8, 128)
    ).astype(ml_dtypes.bfloat16)
    ident = np.eye(128, dtype=np.float32).astype(ml_dtypes.bfloat16)
    e4 = np.zeros((128, 4), np.float32)
    for g in range(4):
        e4[32 * g : 32 * (g + 1), g] = 1.0
    b4 = e4.T.copy()
    ones = np.ones((128, 1), np.float32)
    o1128 = np.ones((1, 128), np.float32)

    in_maps = []
    for i in range(8):
        b, n = i // 4, i % 4
        w = float(routing_weights[b, n])
        lam = float(
            np.exp(np.dot(lq1[n], lk1[n]).astype(np.float32))
            - np.exp(np.dot(lq2[n], lk2[n]).astype(np.float32))
            + np.float32(LAM0)
        )
        wqwk = (wq[n] * wk[n]).astype(np.float32)  # [32]
        whfull = (np.tile(wh[n], H) * (1.0 - LAM0)).astype(np.float32)  # [512]
        wout_f = (Wout[n] * whfull[:, None]).astype(np.float32)
        hT = np.ascontiguousarray(x[b, :, CW * n : CW * (n + 1)].T)
        in_maps.append(
            dict(
                hT=hT,
                hTw=(hT * w).astype(np.float32),
                wqk=np.ascontiguousarray(Wqkv[n][:, : 2 * CW]),
                wv=np.ascontiguousarray(Wqkv[n][:, 2 * CW :]),
                wout=wout_f.astype(ml_dtypes.bfloat16),
                bqk=np.ascontiguousarray(bqkv[n][: 2 * CW].reshape(8, 128, 1)),
                bv=np.ascontiguousarray(bqkv[n][2 * CW :].reshape(1, CW)),
                bout=np.ascontiguousarray(bout_a[n].reshape(4, 128, 1)),
                qmul=np.ascontiguousarray(np.tile(wqwk, 16).reshape(4, 128, 1)),
                wn=np.ascontiguousarray(wn[n].reshape(4, 128, 1)).astype(np.float32),
                lamn=np.full((128, 1), -lam, np.float32),
                epsw2=np.full((1, 1), EPS, np.float32),
                rfscale=np.full((1, 1), 1.0 / (512.0 * w * w), np.float32),
                tri8=tri8,
                ident=ident,
                e4=e4,
                b4=b4,
                o1128=o1128,
                ones=ones,
            )
        )
    return in_maps


def kernel(x, routing_weights, Wqkv, bqkv, Wout, bout, lq1, lk1, lq2, lk2, wq, wk, wh, wn):
    if "nc" not in _cache:
        _cache["nc"] = _build()
    nc = _cache["nc"]

    args = [x, routing_weights, Wqkv, bqkv, Wout, bout, lq1, lk1, lq2, lk2, wq, wk, wh, wn]
    args = [np.asarray(a, np.float32) for a in args]
    in_maps = make_in_maps(*args)

    prof_dir = os.environ.get("KERNEL_PROFILE_DIR")
    if prof_dir:
        res = run_bass_kernel_spmd(
            nc, in_maps, list(range(8)), trace=True, tmpdir=prof_dir
        )
        _cache["exec_time_ns"] = res.exec_time_ns
    else:
        res = run_bass_kernel_spmd(nc, in_maps, list(range(8)))

    out = np.empty((B, L, N_CH * CW), np.float32)
    for i in range(8):
        b, n = i // 4, i % 4
        out[b, :, CW * n : CW * (n + 1)] = res.results[i]["yT"].T
    return out
